# revision 32
# baseline (speedup 1.0000x reference)
"""Multi-head self-attention (B=4, S=2048, D=1024, H=16, RoPE, causal) on 8 trn2 cores.

Sharding: core c -> batch c//2, heads [8*(c%2), 8*(c%2)+8).  Each core computes
its partial output projection out^T [1024, 2048]; host sums the two halves per
batch and transposes back.

v3 design (vs v2 baseline):
- two-rail schedule: PE ~200us (matmuls), ACT ~165us (softmax exp).  Attention
  is ACT-bound per-chunk, so qkproj(u+1) / vproj / oproj(qc-1) matmuls are
  woven into the attention kc loops as "filler" to keep PE busy during exp.
- causal mask moved off PE: scores/exp/AV streams are column-trimmed to the
  causal region; only the leading 128-col diagonal sub-block gets a -1e9
  triangle add on the vector engine (no ident/mask matmuls).
- kc-loop software pipelining: scores(kc+1) issues before AV(kc) so PE never
  waits on exp.
- reciprocal broadcast via SBUF->SBUF DMA (no DRAM bounce).
"""
import sys
sys.path.insert(0, "/opt/trn_rl_repo")
from contextlib import ExitStack
import numpy as np
import ml_dtypes

import concourse.bass as bass
import concourse.bacc as bacc
import concourse.mybir as mybir
from concourse.tile import TileContext
from concourse.bass_utils import run_bass_kernel_spmd

F32 = mybir.dt.float32
BF16 = mybir.dt.bfloat16

B, S, D, H, DK = 4, 2048, 1024, 16, 64
NCORES = 8
NPAIR = 4
QC = 512
NQC = S // QC           # 4
KC = 128
NKC = S // KC           # 16

_BUILT = {}


def _build_nc():
    nc = bacc.Bacc()

    xT_d = nc.declare_dram_parameter("xT", [D, S], BF16, isOutput=False)
    wq_d = nc.declare_dram_parameter("wqT", [D, 512], BF16, isOutput=False)
    wk_d = nc.declare_dram_parameter("wkT", [D, 512], BF16, isOutput=False)
    wv_d = nc.declare_dram_parameter("wvT", [D, 512], BF16, isOutput=False)
    wo_d = nc.declare_dram_parameter("woT", [512, D], BF16, isOutput=False)
    ctab_d = nc.declare_dram_parameter("ctab", [128, S], BF16, isOutput=False)
    stab_d = nc.declare_dram_parameter("stab", [128, S], BF16, isOutput=False)
    tri_d = nc.declare_dram_parameter("trineg", [128, 256], F32, isOutput=False)
    out_d = nc.declare_dram_parameter("outP", [D, S], F32, isOutput=True)

    swapmask = [i ^ 1 for i in range(32)]
    EXP = mybir.ActivationFunctionType.Exp

    with TileContext(nc) as tc, ExitStack() as ctx:
        ep = ctx.enter_context
        consts = ep(tc.tile_pool(name="consts", bufs=1))
        xpool = ep(tc.tile_pool(name="xt", bufs=1))
        wpool = ep(tc.tile_pool(name="w", bufs=1))
        qkpool = ep(tc.tile_pool(name="qk", bufs=1))
        v1pool = ep(tc.tile_pool(name="v1", bufs=1))
        otpool = ep(tc.tile_pool(name="oT", bufs=1))
        shpool = ep(tc.tile_pool(name="sh", bufs=2))
        t1pool = ep(tc.tile_pool(name="t1", bufs=2))
        t2pool = ep(tc.tile_pool(name="t2", bufs=2))
        pqpool = ep(tc.tile_pool(name="pq", bufs=2))
        recBpool = ep(tc.tile_pool(name="recB", bufs=2))
        obpool = ep(tc.tile_pool(name="ob", bufs=2))
        psqpool = ep(tc.tile_pool(name="psq", bufs=2, space="PSUM"))
        psopool = ep(tc.tile_pool(name="pso", bufs=1, space="PSUM"))
        auxpool = ep(tc.tile_pool(name="aux", bufs=2, space="PSUM"))

        # ---------------- input DMA ----------------
        # ordered so the first qkproj units (sc-outer) can start ~5us in:
        # wq, then x in 512-col quarters, weights interleaved
        xts = [xpool.tile([128, S], BF16, tag=f"xt{ic}", name=f"xt{ic}")
               for ic in range(8)]
        wq, wk, wv = [], [], []
        for nm, lst in (("q", wq), ("v", wv), ("k", wk)):
            for ic in range(8):
                lst.append(wpool.tile([128, 512], BF16, tag=f"w{nm}{ic}",
                                      name=f"w{nm}{ic}"))

        def w_dma(lst, src, split=False):
            if split:
                # u=0 column slice of every tile first: the first qkproj
                # matmuls need only wq[ic][:, 0:128]
                for ic in range(8):
                    nc.sync.dma_start(out=lst[ic][:, 0:128],
                                      in_=src[ic * 128:(ic + 1) * 128, 0:128])
                for ic in range(8):
                    nc.sync.dma_start(out=lst[ic][:, 128:],
                                      in_=src[ic * 128:(ic + 1) * 128, 128:])
            else:
                for ic in range(8):
                    nc.sync.dma_start(out=lst[ic],
                                      in_=src[ic * 128:(ic + 1) * 128, :])

        def x_dma(q, split=False):
            halves = ((slice(q * 512, q * 512 + 256),
                       slice(q * 512 + 256, (q + 1) * 512))
                      if split else (slice(q * 512, (q + 1) * 512),))
            for csl in halves:
                for ic in range(8):
                    nc.sync.dma_start(out=xts[ic][:, csl],
                                      in_=xT_d[ic * 128:(ic + 1) * 128, csl])

        ctab = consts.tile([128, S], BF16, tag="ctab")
        stab = consts.tile([128, S], BF16, tag="stab")
        trineg = consts.tile([128, 256], F32, tag="trineg")

        def tab_dma(q):
            csl = slice(q * 512, (q + 1) * 512)
            nc.sync.dma_start(out=ctab[:, csl], in_=ctab_d[:, csl])
            nc.sync.dma_start(out=stab[:, csl], in_=stab_d[:, csl])

        w_dma(wq, wq_d)
        for ic in range(8):
            nc.sync.dma_start(out=xts[ic], in_=xT_d[ic * 128:(ic + 1) * 128, :])
        w_dma(wk, wk_d)
        for q in range(4):
            tab_dma(q)
        nc.sync.dma_start(out=trineg, in_=tri_d[:, :])
        w_dma(wv, wv_d)
        wo = []
        for u in range(NPAIR):
            w = wpool.tile([128, D], BF16, tag=f"wo{u}")
            nc.sync.dma_start(out=w, in_=wo_d[u * 128:(u + 1) * 128, :])
            wo.append(w)
        trineg3 = trineg.rearrange("p (h q) -> p h q", q=128)

        # warm up the ACT exp table early so the ~2.7us load is off the rail
        warm = consts.tile([1, 8], F32, tag="warm")
        nc.scalar.activation(out=warm, in_=trineg[0:1, 0:8], func=EXP,
                             scale=0.0)

        # persistent tiles
        v1 = v1pool.tile([128, NKC, 8, 65], BF16, tag="v1")
        nc.vector.memset(v1[:, :, :, 64:65], 1.0)
        qTs, kTs = [], []
        for u in range(NPAIR):
            qTs.append(qkpool.tile([128, S], BF16, tag=f"qT{u}", name=f"qT{u}"))
            kTs.append(qkpool.tile([128, S], BF16, tag=f"kT{u}", name=f"kT{u}"))
        oTs = [otpool.tile([128, S], BF16, tag=f"oT{u}", name=f"oT{u}")
               for u in range(NPAIR)]
        drpool = ep(tc.tile_pool(name="dr", bufs=4, space="DRAM"))
        dnrowpool = ep(tc.tile_pool(name="dnrow", bufs=2))
        dnBpool = ep(tc.tile_pool(name="dnB", bufs=2))
        onesc = consts.tile([64, 1024], F32, tag="onesc")
        nc.vector.memset(onesc, 1.0)

        # ---------------- emission helpers ----------------
        def vproj_chunk(sb, fine=False):
            """one sb block (128 seq positions) of the V projection"""
            aux = auxpool.tile([128, 512], F32, tag="aux")
            for ic in range(8):
                nc.tensor.matmul(aux, xts[ic][:, sb * 128:(sb + 1) * 128],
                                 wv[ic], start=(ic == 0), stop=(ic == 7),
                                 skip_group_check=True)
            nc.vector.tensor_copy(
                out=v1[:, sb, :, 0:64],
                in_=aux.rearrange("p (h d) -> p h d", d=64))
            if fine:
                yield

        def qkproj_unit(u, pj, sc):
            """one (proj, 512-col seq chunk): 8 matmuls + RoPE"""
            wlist, dst = ((wq, qTs[u]) if pj == 0 else (wk, kTs[u]))
            ssl = slice(sc * 512, (sc + 1) * 512)
            aux = auxpool.tile([128, 512], F32, tag="aux")
            for ic in range(8):
                nc.tensor.matmul(aux, wlist[ic][:, u * 128:(u + 1) * 128],
                                 xts[ic][:, ssl], start=(ic == 0),
                                 stop=(ic == 7), skip_group_check=True)
                if ic in (1, 3, 5):
                    yield
            sh = shpool.tile([128, 512], F32, tag="sh")
            nc.vector.stream_shuffle(out=sh, in_=aux, mask=swapmask)
            t1 = t1pool.tile([128, 512], BF16, tag="t1")
            nc.vector.tensor_mul(out=t1, in0=aux, in1=ctab[:, ssl])
            t2 = t2pool.tile([128, 512], BF16, tag="t2")
            nc.gpsimd.tensor_mul(out=t2, in0=sh, in1=stab[:, ssl])
            nc.gpsimd.tensor_add(out=dst[:, ssl], in0=t1, in1=t2)
            yield

        def qkproj_gen(u):
            for sc in range(4):
                for pj in range(2):
                    yield from qkproj_unit(u, pj, sc)

        def vproj_gen(sb0, sb1):
            for sb in range(sb0, sb1):
                yield from vproj_chunk(sb, fine=True)

        def oproj_gen(qc):
            """output projection for q chunk qc: 8 (ot,half) groups"""
            qsl = slice(qc * QC, (qc + 1) * QC)
            for oc in range(8):
                po = auxpool.tile([128, 512], F32, tag="aux")
                for u in range(NPAIR):
                    nc.tensor.matmul(
                        po, wo[u][:, oc * 128:(oc + 1) * 128],
                        oTs[u][:, qsl], start=(u == 0), stop=(u == NPAIR - 1),
                        skip_group_check=True)
                    if u == 1:
                        yield
                ob = obpool.tile([128, 512], F32, tag="ob")
                nc.vector.tensor_copy(out=ob, in_=po)
                nc.sync.dma_start(out=out_d[oc * 128:(oc + 1) * 128, qsl],
                                  in_=ob)
                yield

        def attn_chunk(u, qc, filler, npull=1):
            nact = 4 * qc + 4
            qbase = qc * QC
            pso = psopool.tile([128, 1024], F32, tag="pso")
            prev = None

            def emit_av(kc, pq):
                off = max(kc - 4 * qc, 0) * 128
                for h in range(2):
                    nc.tensor.matmul(
                        pso[0:65, h * 512 + off:(h + 1) * 512],
                        v1[:, kc, 2 * u + h, :],
                        pq[:, h * 512 + off:(h + 1) * 512],
                        start=(kc == 0), stop=(kc == nact - 1),
                        skip_group_check=True)

            for kc in range(nact):
                moff = kc - 4 * qc
                off = max(moff, 0) * 128
                psq = psqpool.tile([128, 1024], F32, tag="psq")
                for h in range(2):
                    nc.tensor.matmul(
                        psq[:, h * 512 + off:(h + 1) * 512],
                        kTs[u][h * 64:(h + 1) * 64, kc * 128:(kc + 1) * 128],
                        qTs[u][h * 64:(h + 1) * 64, qbase + off:qbase + 512],
                        start=True, stop=True, skip_group_check=True)
                psq3 = psq.rearrange("p (h q) -> p h q", q=512)
                if moff >= 0:
                    nc.vector.tensor_add(out=psq3[:, :, off:off + 128],
                                         in0=psq3[:, :, off:off + 128],
                                         in1=trineg3)
                pq = pqpool.tile([128, 1024], BF16, tag="pq")
                pq3 = pq.rearrange("p (h q) -> p h q", q=512)
                nc.scalar.activation(out=pq3[:, :, off:], in_=psq3[:, :, off:],
                                     func=EXP, scale=0.125)
                if prev is not None:
                    emit_av(*prev)
                for _ in range(npull):
                    next(filler, None)
                prev = (kc, pq)
            emit_av(*prev)
            # cross-partition moves must be TensorTensor ops (BIR verifier
            # rejects TensorCopy with mismatched start partitions); gpsimd
            # carries these so the vector engine keeps slack
            for h in range(2):
                nc.vector.tensor_mul(
                    out=oTs[u][h * 64:(h + 1) * 64, qbase:qbase + 512],
                    in0=pso[0:64, h * 512:(h + 1) * 512],
                    in1=onesc[:, 0:512])
            dnrow = dnrowpool.tile([1, 1024], F32, tag="dnrow")
            nc.vector.tensor_mul(out=dnrow, in0=pso[64:65, :],
                                 in1=onesc[0:1, :])
            # distributed softmax denominator: bounce the dn row through
            # DRAM to broadcast it across partitions, then reciprocal + two
            # in-place normalize multiplies.  No cross-u barrier.
            drt = drpool.tile([1, 1024], F32)
            nc.sync.dma_start(out=drt, in_=dnrow)
            dnB = dnBpool.tile([128, 1024], F32, tag="dnB")
            nc.sync.dma_start(out=dnB, in_=drt[0:1, :].to_broadcast((128, 1024)))
            recB = recBpool.tile([128, 1024], F32, tag="recB")
            nc.vector.reciprocal_approx_fast(out=recB, in_=dnB)
            for h in range(2):
                hs = slice(h * 64, (h + 1) * 64)
                nc.gpsimd.tensor_mul(
                    out=oTs[u][hs, qbase:qbase + 512],
                    in0=oTs[u][hs, qbase:qbase + 512],
                    in1=recB[hs, h * 512:(h + 1) * 512])

        def chain(*gens):
            for g in gens:
                yield from g

        def drain(g):
            for _ in g:
                pass

        # ---------------- schedule ----------------
        _sc = nc.named_scope("qkproj0"); _sc.__enter__()
        drain(qkproj_gen(0))
        for sb in range(4):
            for _ in vproj_chunk(sb):
                pass
        _sc.__exit__(None, None, None)

        fillers = [chain(vproj_gen(4, NKC), qkproj_gen(1)),
                   qkproj_gen(2), qkproj_gen(3)]
        for u in range(NPAIR):
            _sc = nc.named_scope(f"stripe{u}"); _sc.__enter__()
            if u < 3:
                filler = fillers[u]
                for qc in range(NQC):
                    attn_chunk(u, qc, filler)
                drain(filler)
            else:
                filler = iter(())
                for qc in range(NQC):
                    attn_chunk(3, qc, filler)
                    if qc < NQC - 1:
                        filler = chain(filler, oproj_gen(qc))
                drain(filler)
            _sc.__exit__(None, None, None)
        _sc = nc.named_scope("oproj3"); _sc.__enter__()
        drain(oproj_gen(NQC - 1))
        _sc.__exit__(None, None, None)

    nc.compile()
    return nc


def get_nc():
    if "nc" not in _BUILT:
        _BUILT["nc"] = _build_nc()
    return _BUILT["nc"]


def _host_prep(x, Wq, Wk, Wv, Wo, token_positions):
    pos = np.asarray(token_positions).astype(np.float32)
    half = DK // 2
    inv_freq = 1.0 / (10000.0 ** (np.arange(half, dtype=np.float32) * 2.0 / DK))
    ang = pos[:, None] * inv_freq[None, :]          # [S, 32]
    cos = np.cos(ang).astype(np.float32)            # [S, 32]
    sin = np.sin(ang).astype(np.float32)
    p = np.arange(128)
    j = (p % 64) // 2
    sign = np.where(p % 2 == 0, -1.0, 1.0).astype(np.float32)
    bf = ml_dtypes.bfloat16
    ctab = np.ascontiguousarray(cos[:, j].T).astype(bf)           # [128, S]
    stab = np.ascontiguousarray(sin[:, j].T * sign[:, None]).astype(bf)

    kk = np.arange(128)[:, None]
    qq = np.arange(128)[None, :]
    tri = np.where(qq >= kk, 0.0, -1e9).astype(np.float32)        # [128,128]
    trineg = np.concatenate([tri, tri], axis=1)                   # [128,256]

    in_maps = []
    for c in range(NCORES):
        b, hf = divmod(c, 2)
        m = {}
        m["xT"] = np.ascontiguousarray(x[b].T).astype(bf)
        m["wqT"] = np.ascontiguousarray(Wq[hf * 512:(hf + 1) * 512, :].T).astype(bf)
        m["wkT"] = np.ascontiguousarray(Wk[hf * 512:(hf + 1) * 512, :].T).astype(bf)
        m["wvT"] = np.ascontiguousarray(Wv[hf * 512:(hf + 1) * 512, :].T).astype(bf)
        m["woT"] = np.ascontiguousarray(Wo[:, hf * 512:(hf + 1) * 512].T).astype(bf)
        m["ctab"] = ctab
        m["stab"] = stab
        m["trineg"] = trineg
        in_maps.append(m)
    return in_maps


def run(inputs, trace=False, **kw):
    in_maps = _host_prep(**{k: np.asarray(v) for k, v in inputs.items()})
    nc = get_nc()
    res = run_bass_kernel_spmd(nc, in_maps, list(range(NCORES)), trace=trace, **kw)
    outs = [res.results[c]["outP"] for c in range(NCORES)]
    out = np.stack([(outs[2 * b] + outs[2 * b + 1]).T for b in range(B)])
    return out.astype(np.float32), res


def kernel(**inputs):
    out, _ = run(inputs, trace=False)
    return out


# revision 33
# speedup vs baseline: 1.1658x; 1.1658x over previous
"""Multi-head self-attention (B=4, S=2048, D=1024, H=16, RoPE, causal) on 8 trn2 cores.

Sharding: core c -> batch c//2, heads [8*(c%2), 8*(c%2)+8).  Each core computes
its partial output projection out^T [1024, 2048]; host sums the two halves per
batch and transposes back.

v3 design (vs v2 baseline):
- two-rail schedule: PE ~200us (matmuls), ACT ~165us (softmax exp).  Attention
  is ACT-bound per-chunk, so qkproj(u+1) / vproj / oproj(qc-1) matmuls are
  woven into the attention kc loops as "filler" to keep PE busy during exp.
- causal mask moved off PE: scores/exp/AV streams are column-trimmed to the
  causal region; only the leading 128-col diagonal sub-block gets a -1e9
  triangle add on the vector engine (no ident/mask matmuls).
- kc-loop software pipelining: scores(kc+1) issues before AV(kc) so PE never
  waits on exp.
- reciprocal broadcast via SBUF->SBUF DMA (no DRAM bounce).
"""
import sys
sys.path.insert(0, "/opt/trn_rl_repo")
from contextlib import ExitStack
import numpy as np
import ml_dtypes

import concourse.bass as bass
import concourse.bacc as bacc
import concourse.mybir as mybir
from concourse.tile import TileContext
from concourse.bass_utils import run_bass_kernel_spmd

F32 = mybir.dt.float32
BF16 = mybir.dt.bfloat16

B, S, D, H, DK = 4, 2048, 1024, 16, 64
NCORES = 8
NPAIR = 4
QC = 512
NQC = S // QC           # 4
KC = 128
NKC = S // KC           # 16

_BUILT = {}


def _build_nc():
    nc = bacc.Bacc()

    xT_d = nc.declare_dram_parameter("xT", [D, S], BF16, isOutput=False)
    wq_d = nc.declare_dram_parameter("wqT", [D, 512], BF16, isOutput=False)
    wk_d = nc.declare_dram_parameter("wkT", [D, 512], BF16, isOutput=False)
    wv_d = nc.declare_dram_parameter("wvT", [D, 512], BF16, isOutput=False)
    wo_d = nc.declare_dram_parameter("woT", [512, D], BF16, isOutput=False)
    ctab_d = nc.declare_dram_parameter("ctab", [128, S], BF16, isOutput=False)
    stab_d = nc.declare_dram_parameter("stab", [128, S], BF16, isOutput=False)
    tri_d = nc.declare_dram_parameter("trineg", [128, 256], F32, isOutput=False)
    out_d = nc.declare_dram_parameter("outP", [D, S], F32, isOutput=True)

    swapmask = [i ^ 1 for i in range(32)]
    EXP = mybir.ActivationFunctionType.Exp

    with TileContext(nc) as tc, ExitStack() as ctx:
        ep = ctx.enter_context
        consts = ep(tc.tile_pool(name="consts", bufs=1))
        xpool = ep(tc.tile_pool(name="xt", bufs=1))
        wpool = ep(tc.tile_pool(name="w", bufs=1))
        qkpool = ep(tc.tile_pool(name="qk", bufs=1))
        v1pool = ep(tc.tile_pool(name="v1", bufs=1))
        otpool = ep(tc.tile_pool(name="oT", bufs=1))
        shpool = ep(tc.tile_pool(name="sh", bufs=2))
        t1pool = ep(tc.tile_pool(name="t1", bufs=2))
        t2pool = ep(tc.tile_pool(name="t2", bufs=2))
        pqpool = ep(tc.tile_pool(name="pq", bufs=2))
        recBpool = ep(tc.tile_pool(name="recB", bufs=2))
        obpool = ep(tc.tile_pool(name="ob", bufs=2))
        psqpool = ep(tc.tile_pool(name="psq", bufs=2, space="PSUM"))
        psopool = ep(tc.tile_pool(name="pso", bufs=1, space="PSUM"))
        auxpool = ep(tc.tile_pool(name="aux", bufs=2, space="PSUM"))

        # ---------------- input DMA ----------------
        # ordered so the first qkproj units (sc-outer) can start ~5us in:
        # wq, then x in 512-col quarters, weights interleaved
        xts = [xpool.tile([128, S], BF16, tag=f"xt{ic}", name=f"xt{ic}")
               for ic in range(8)]
        wq, wk, wv = [], [], []
        for nm, lst in (("q", wq), ("v", wv), ("k", wk)):
            for ic in range(8):
                lst.append(wpool.tile([128, 512], BF16, tag=f"w{nm}{ic}",
                                      name=f"w{nm}{ic}"))

        def w_dma(lst, src, split=False):
            if split:
                # u=0 column slice of every tile first: the first qkproj
                # matmuls need only wq[ic][:, 0:128]
                for ic in range(8):
                    nc.sync.dma_start(out=lst[ic][:, 0:128],
                                      in_=src[ic * 128:(ic + 1) * 128, 0:128])
                for ic in range(8):
                    nc.sync.dma_start(out=lst[ic][:, 128:],
                                      in_=src[ic * 128:(ic + 1) * 128, 128:])
            else:
                for ic in range(8):
                    nc.sync.dma_start(out=lst[ic],
                                      in_=src[ic * 128:(ic + 1) * 128, :])

        def x_dma(q, split=False):
            halves = ((slice(q * 512, q * 512 + 256),
                       slice(q * 512 + 256, (q + 1) * 512))
                      if split else (slice(q * 512, (q + 1) * 512),))
            for csl in halves:
                for ic in range(8):
                    nc.sync.dma_start(out=xts[ic][:, csl],
                                      in_=xT_d[ic * 128:(ic + 1) * 128, csl])

        ctab = consts.tile([128, S], BF16, tag="ctab")
        stab = consts.tile([128, S], BF16, tag="stab")
        trineg = consts.tile([128, 256], F32, tag="trineg")

        def tab_dma(q):
            csl = slice(q * 512, (q + 1) * 512)
            nc.sync.dma_start(out=ctab[:, csl], in_=ctab_d[:, csl])
            nc.sync.dma_start(out=stab[:, csl], in_=stab_d[:, csl])

        w_dma(wq, wq_d)
        x_dma(0)
        w_dma(wv, wv_d)
        w_dma(wk, wk_d)
        tab_dma(0)
        nc.sync.dma_start(out=trineg, in_=tri_d[:, :])
        x_dma(1)
        tab_dma(1)
        x_dma(2)
        tab_dma(2)
        x_dma(3)
        tab_dma(3)
        wo = []
        for u in range(NPAIR):
            w = wpool.tile([128, D], BF16, tag=f"wo{u}")
            nc.sync.dma_start(out=w, in_=wo_d[u * 128:(u + 1) * 128, :])
            wo.append(w)
        trineg3 = trineg.rearrange("p (h q) -> p h q", q=128)

        # warm up the ACT exp table early so the ~2.7us load is off the rail
        warm = consts.tile([1, 8], F32, tag="warm")
        nc.scalar.activation(out=warm, in_=trineg[0:1, 0:8], func=EXP,
                             scale=0.0)

        # persistent tiles
        v1 = v1pool.tile([128, NKC, 8, 65], BF16, tag="v1")
        nc.vector.memset(v1[:, :, :, 64:65], 1.0)
        qTs, kTs = [], []
        for u in range(NPAIR):
            qTs.append(qkpool.tile([128, S], BF16, tag=f"qT{u}", name=f"qT{u}"))
            kTs.append(qkpool.tile([128, S], BF16, tag=f"kT{u}", name=f"kT{u}"))
        oTs = [otpool.tile([128, S], BF16, tag=f"oT{u}", name=f"oT{u}")
               for u in range(NPAIR)]
        drpool = ep(tc.tile_pool(name="dr", bufs=4, space="DRAM"))
        dnrowpool = ep(tc.tile_pool(name="dnrow", bufs=2))
        dnBpool = ep(tc.tile_pool(name="dnB", bufs=2))
        onesc = consts.tile([64, 1024], F32, tag="onesc")
        nc.vector.memset(onesc, 1.0)

        # ---------------- emission helpers ----------------
        def vproj_chunk(sb, fine=False):
            """one sb block (128 seq positions) of the V projection"""
            aux = auxpool.tile([128, 512], F32, tag="aux")
            for ic in range(8):
                nc.tensor.matmul(aux, xts[ic][:, sb * 128:(sb + 1) * 128],
                                 wv[ic], start=(ic == 0), stop=(ic == 7),
                                 skip_group_check=True)
            nc.vector.tensor_copy(
                out=v1[:, sb, :, 0:64],
                in_=aux.rearrange("p (h d) -> p h d", d=64))
            if fine:
                yield

        def qkproj_unit(u, pj, sc):
            """one (proj, 512-col seq chunk): 8 matmuls + RoPE"""
            wlist, dst = ((wq, qTs[u]) if pj == 0 else (wk, kTs[u]))
            ssl = slice(sc * 512, (sc + 1) * 512)
            aux = auxpool.tile([128, 512], F32, tag="aux")
            for ic in range(8):
                nc.tensor.matmul(aux, wlist[ic][:, u * 128:(u + 1) * 128],
                                 xts[ic][:, ssl], start=(ic == 0),
                                 stop=(ic == 7), skip_group_check=True)
                if ic in (1, 3, 5):
                    yield
            sh = shpool.tile([128, 512], F32, tag="sh")
            nc.vector.stream_shuffle(out=sh, in_=aux, mask=swapmask)
            t1 = t1pool.tile([128, 512], BF16, tag="t1")
            nc.vector.tensor_mul(out=t1, in0=aux, in1=ctab[:, ssl])
            t2 = t2pool.tile([128, 512], BF16, tag="t2")
            nc.gpsimd.tensor_mul(out=t2, in0=sh, in1=stab[:, ssl])
            nc.gpsimd.tensor_add(out=dst[:, ssl], in0=t1, in1=t2)
            yield

        def qkproj_gen(u):
            for sc in range(4):
                for pj in range(2):
                    yield from qkproj_unit(u, pj, sc)

        def vproj_gen(sb0, sb1):
            for sb in range(sb0, sb1):
                yield from vproj_chunk(sb, fine=True)

        def oproj_gen(qc):
            """output projection for q chunk qc: 8 (ot,half) groups"""
            qsl = slice(qc * QC, (qc + 1) * QC)
            for oc in range(8):
                po = auxpool.tile([128, 512], F32, tag="aux")
                for u in range(NPAIR):
                    nc.tensor.matmul(
                        po, wo[u][:, oc * 128:(oc + 1) * 128],
                        oTs[u][:, qsl], start=(u == 0), stop=(u == NPAIR - 1),
                        skip_group_check=True)
                    if u == 1:
                        yield
                ob = obpool.tile([128, 512], F32, tag="ob")
                nc.vector.tensor_copy(out=ob, in_=po)
                nc.sync.dma_start(out=out_d[oc * 128:(oc + 1) * 128, qsl],
                                  in_=ob)
                yield

        def attn_chunk(u, qc, filler, npull=1):
            nact = 4 * qc + 4
            qbase = qc * QC
            pso = psopool.tile([128, 1024], F32, tag="pso")
            prev = None

            def emit_av(kc, pq):
                off = max(kc - 4 * qc, 0) * 128
                for h in range(2):
                    nc.tensor.matmul(
                        pso[0:65, h * 512 + off:(h + 1) * 512],
                        v1[:, kc, 2 * u + h, :],
                        pq[:, h * 512 + off:(h + 1) * 512],
                        start=(kc == 0), stop=(kc == nact - 1),
                        skip_group_check=True)

            for kc in range(nact):
                moff = kc - 4 * qc
                off = max(moff, 0) * 128
                psq = psqpool.tile([128, 1024], F32, tag="psq")
                for h in range(2):
                    nc.tensor.matmul(
                        psq[:, h * 512 + off:(h + 1) * 512],
                        kTs[u][h * 64:(h + 1) * 64, kc * 128:(kc + 1) * 128],
                        qTs[u][h * 64:(h + 1) * 64, qbase + off:qbase + 512],
                        start=True, stop=True, skip_group_check=True)
                psq3 = psq.rearrange("p (h q) -> p h q", q=512)
                if moff >= 0:
                    nc.vector.tensor_add(out=psq3[:, :, off:off + 128],
                                         in0=psq3[:, :, off:off + 128],
                                         in1=trineg3)
                pq = pqpool.tile([128, 1024], BF16, tag="pq")
                pq3 = pq.rearrange("p (h q) -> p h q", q=512)
                nc.scalar.activation(out=pq3[:, :, off:], in_=psq3[:, :, off:],
                                     func=EXP, scale=0.125)
                if prev is not None:
                    emit_av(*prev)
                for _ in range(npull):
                    next(filler, None)
                prev = (kc, pq)
            emit_av(*prev)
            # cross-partition moves must be TensorTensor ops (BIR verifier
            # rejects TensorCopy with mismatched start partitions); gpsimd
            # carries these so the vector engine keeps slack
            for h in range(2):
                nc.vector.tensor_mul(
                    out=oTs[u][h * 64:(h + 1) * 64, qbase:qbase + 512],
                    in0=pso[0:64, h * 512:(h + 1) * 512],
                    in1=onesc[:, 0:512])
            dnrow = dnrowpool.tile([1, 1024], F32, tag="dnrow")
            nc.vector.tensor_mul(out=dnrow, in0=pso[64:65, :],
                                 in1=onesc[0:1, :])
            # distributed softmax denominator: bounce the dn row through
            # DRAM to broadcast it across partitions, then reciprocal + two
            # in-place normalize multiplies.  No cross-u barrier.
            drt = drpool.tile([1, 1024], F32)
            nc.sync.dma_start(out=drt, in_=dnrow)
            dnB = dnBpool.tile([128, 1024], F32, tag="dnB")
            nc.sync.dma_start(out=dnB, in_=drt[0:1, :].to_broadcast((128, 1024)))
            recB = recBpool.tile([128, 1024], F32, tag="recB")
            nc.vector.reciprocal_approx_fast(out=recB, in_=dnB)
            for h in range(2):
                hs = slice(h * 64, (h + 1) * 64)
                nc.gpsimd.tensor_mul(
                    out=oTs[u][hs, qbase:qbase + 512],
                    in0=oTs[u][hs, qbase:qbase + 512],
                    in1=recB[hs, h * 512:(h + 1) * 512])

        def chain(*gens):
            for g in gens:
                yield from g

        def drain(g):
            for _ in g:
                pass

        # ---------------- schedule ----------------
        _sc = nc.named_scope("qkproj0"); _sc.__enter__()
        for sc in range(4):
            for pj in range(2):
                drain(qkproj_unit(0, pj, sc))
            if sc == 0:
                for sb in range(4):
                    for _ in vproj_chunk(sb):
                        pass
        _sc.__exit__(None, None, None)

        fillers = [chain(vproj_gen(4, NKC), qkproj_gen(1)),
                   qkproj_gen(2), qkproj_gen(3)]
        for u in range(NPAIR):
            _sc = nc.named_scope(f"stripe{u}"); _sc.__enter__()
            if u < 3:
                filler = fillers[u]
                for qc in range(NQC):
                    attn_chunk(u, qc, filler)
                drain(filler)
            else:
                filler = iter(())
                for qc in range(NQC):
                    attn_chunk(3, qc, filler)
                    if qc < NQC - 1:
                        filler = chain(filler, oproj_gen(qc))
                drain(filler)
            _sc.__exit__(None, None, None)
        _sc = nc.named_scope("oproj3"); _sc.__enter__()
        drain(oproj_gen(NQC - 1))
        _sc.__exit__(None, None, None)

    nc.compile()
    return nc


def get_nc():
    if "nc" not in _BUILT:
        _BUILT["nc"] = _build_nc()
    return _BUILT["nc"]


def _host_prep(x, Wq, Wk, Wv, Wo, token_positions):
    pos = np.asarray(token_positions).astype(np.float32)
    half = DK // 2
    inv_freq = 1.0 / (10000.0 ** (np.arange(half, dtype=np.float32) * 2.0 / DK))
    ang = pos[:, None] * inv_freq[None, :]          # [S, 32]
    cos = np.cos(ang).astype(np.float32)            # [S, 32]
    sin = np.sin(ang).astype(np.float32)
    p = np.arange(128)
    j = (p % 64) // 2
    sign = np.where(p % 2 == 0, -1.0, 1.0).astype(np.float32)
    bf = ml_dtypes.bfloat16
    ctab = np.ascontiguousarray(cos[:, j].T).astype(bf)           # [128, S]
    stab = np.ascontiguousarray(sin[:, j].T * sign[:, None]).astype(bf)

    kk = np.arange(128)[:, None]
    qq = np.arange(128)[None, :]
    tri = np.where(qq >= kk, 0.0, -1e9).astype(np.float32)        # [128,128]
    trineg = np.concatenate([tri, tri], axis=1)                   # [128,256]

    in_maps = []
    for c in range(NCORES):
        b, hf = divmod(c, 2)
        m = {}
        m["xT"] = np.ascontiguousarray(x[b].T).astype(bf)
        m["wqT"] = np.ascontiguousarray(Wq[hf * 512:(hf + 1) * 512, :].T).astype(bf)
        m["wkT"] = np.ascontiguousarray(Wk[hf * 512:(hf + 1) * 512, :].T).astype(bf)
        m["wvT"] = np.ascontiguousarray(Wv[hf * 512:(hf + 1) * 512, :].T).astype(bf)
        m["woT"] = np.ascontiguousarray(Wo[:, hf * 512:(hf + 1) * 512].T).astype(bf)
        m["ctab"] = ctab
        m["stab"] = stab
        m["trineg"] = trineg
        in_maps.append(m)
    return in_maps


def run(inputs, trace=False, **kw):
    in_maps = _host_prep(**{k: np.asarray(v) for k, v in inputs.items()})
    nc = get_nc()
    res = run_bass_kernel_spmd(nc, in_maps, list(range(NCORES)), trace=trace, **kw)
    outs = [res.results[c]["outP"] for c in range(NCORES)]
    out = np.stack([(outs[2 * b] + outs[2 * b + 1]).T for b in range(B)])
    return out.astype(np.float32), res


def kernel(**inputs):
    out, _ = run(inputs, trace=False)
    return out


# revision 34
# speedup vs baseline: 1.1782x; 1.0106x over previous
"""Multi-head self-attention (B=4, S=2048, D=1024, H=16, RoPE, causal) on 8 trn2 cores.

Sharding: core c -> batch c//2, heads [8*(c%2), 8*(c%2)+8).  Each core computes
its partial output projection out^T [1024, 2048]; host sums the two halves per
batch and transposes back.

v3 design (vs v2 baseline):
- two-rail schedule: PE ~200us (matmuls), ACT ~165us (softmax exp).  Attention
  is ACT-bound per-chunk, so qkproj(u+1) / vproj / oproj(qc-1) matmuls are
  woven into the attention kc loops as "filler" to keep PE busy during exp.
- causal mask moved off PE: scores/exp/AV streams are column-trimmed to the
  causal region; only the leading 128-col diagonal sub-block gets a -1e9
  triangle add on the vector engine (no ident/mask matmuls).
- kc-loop software pipelining: scores(kc+1) issues before AV(kc) so PE never
  waits on exp.
- reciprocal broadcast via SBUF->SBUF DMA (no DRAM bounce).
"""
import sys
sys.path.insert(0, "/opt/trn_rl_repo")
from contextlib import ExitStack
import numpy as np
import ml_dtypes

import concourse.bass as bass
import concourse.bacc as bacc
import concourse.mybir as mybir
from concourse.tile import TileContext
from concourse.bass_utils import run_bass_kernel_spmd

F32 = mybir.dt.float32
BF16 = mybir.dt.bfloat16

B, S, D, H, DK = 4, 2048, 1024, 16, 64
NCORES = 8
NPAIR = 4
QC = 512
NQC = S // QC           # 4
KC = 128
NKC = S // KC           # 16

_BUILT = {}


def _build_nc():
    nc = bacc.Bacc()

    xT_d = nc.declare_dram_parameter("xT", [D, S], BF16, isOutput=False)
    wq_d = nc.declare_dram_parameter("wqT", [D, 512], BF16, isOutput=False)
    wk_d = nc.declare_dram_parameter("wkT", [D, 512], BF16, isOutput=False)
    wv_d = nc.declare_dram_parameter("wvT", [D, 512], BF16, isOutput=False)
    wo_d = nc.declare_dram_parameter("woT", [512, D], BF16, isOutput=False)
    ctab_d = nc.declare_dram_parameter("ctab", [128, S], BF16, isOutput=False)
    stab_d = nc.declare_dram_parameter("stab", [128, S], BF16, isOutput=False)
    tri_d = nc.declare_dram_parameter("trineg", [128, 256], F32, isOutput=False)
    out_d = nc.declare_dram_parameter("outP", [D, S], F32, isOutput=True)

    swapmask = [i ^ 1 for i in range(32)]
    EXP = mybir.ActivationFunctionType.Exp

    with TileContext(nc) as tc, ExitStack() as ctx:
        ep = ctx.enter_context
        consts = ep(tc.tile_pool(name="consts", bufs=1))
        xpool = ep(tc.tile_pool(name="xt", bufs=1))
        wpool = ep(tc.tile_pool(name="w", bufs=1))
        qkpool = ep(tc.tile_pool(name="qk", bufs=1))
        v1pool = ep(tc.tile_pool(name="v1", bufs=1))
        otpool = ep(tc.tile_pool(name="oT", bufs=1))
        shpool = ep(tc.tile_pool(name="sh", bufs=2))
        t1pool = ep(tc.tile_pool(name="t1", bufs=2))
        t2pool = ep(tc.tile_pool(name="t2", bufs=2))
        pqpool = ep(tc.tile_pool(name="pq", bufs=2))
        recBpool = ep(tc.tile_pool(name="recB", bufs=2))
        obpool = ep(tc.tile_pool(name="ob", bufs=2))
        psqpool = ep(tc.tile_pool(name="psq", bufs=2, space="PSUM"))
        psopool = ep(tc.tile_pool(name="pso", bufs=1, space="PSUM"))
        auxpool = ep(tc.tile_pool(name="aux", bufs=2, space="PSUM"))

        # ---------------- input DMA ----------------
        # ordered so the first qkproj units (sc-outer) can start ~5us in:
        # wq, then x in 512-col quarters, weights interleaved
        xts = [xpool.tile([128, S], BF16, tag=f"xt{ic}", name=f"xt{ic}")
               for ic in range(8)]
        wq, wk, wv = [], [], []
        for nm, lst in (("q", wq), ("v", wv), ("k", wk)):
            for ic in range(8):
                lst.append(wpool.tile([128, 512], BF16, tag=f"w{nm}{ic}",
                                      name=f"w{nm}{ic}"))

        def w_dma(lst, src, split=False):
            if split:
                # u=0 column slice of every tile first: the first qkproj
                # matmuls need only wq[ic][:, 0:128]
                for ic in range(8):
                    nc.sync.dma_start(out=lst[ic][:, 0:128],
                                      in_=src[ic * 128:(ic + 1) * 128, 0:128])
                for ic in range(8):
                    nc.sync.dma_start(out=lst[ic][:, 128:],
                                      in_=src[ic * 128:(ic + 1) * 128, 128:])
            else:
                for ic in range(8):
                    nc.sync.dma_start(out=lst[ic],
                                      in_=src[ic * 128:(ic + 1) * 128, :])

        def x_dma(q, split=False):
            halves = ((slice(q * 512, q * 512 + 256),
                       slice(q * 512 + 256, (q + 1) * 512))
                      if split else (slice(q * 512, (q + 1) * 512),))
            for csl in halves:
                for ic in range(8):
                    nc.sync.dma_start(out=xts[ic][:, csl],
                                      in_=xT_d[ic * 128:(ic + 1) * 128, csl])

        ctab = consts.tile([128, S], BF16, tag="ctab")
        stab = consts.tile([128, S], BF16, tag="stab")
        trineg = consts.tile([128, 256], F32, tag="trineg")

        def tab_dma(q):
            csl = slice(q * 512, (q + 1) * 512)
            nc.sync.dma_start(out=ctab[:, csl], in_=ctab_d[:, csl])
            nc.sync.dma_start(out=stab[:, csl], in_=stab_d[:, csl])

        w_dma(wq, wq_d)
        x_dma(0)
        w_dma(wv, wv_d)
        w_dma(wk, wk_d)
        tab_dma(0)
        nc.sync.dma_start(out=trineg, in_=tri_d[:, :])
        x_dma(1)
        tab_dma(1)
        x_dma(2)
        tab_dma(2)
        x_dma(3)
        tab_dma(3)
        wo = []
        for u in range(NPAIR):
            w = wpool.tile([128, D], BF16, tag=f"wo{u}")
            nc.sync.dma_start(out=w, in_=wo_d[u * 128:(u + 1) * 128, :])
            wo.append(w)
        trineg3 = trineg.rearrange("p (h q) -> p h q", q=128)

        # warm up the ACT exp table early so the ~2.7us load is off the rail
        warm = consts.tile([1, 8], F32, tag="warm")
        nc.scalar.activation(out=warm, in_=trineg[0:1, 0:8], func=EXP,
                             scale=0.0)

        # persistent tiles
        v1 = v1pool.tile([128, NKC, 8, 65], BF16, tag="v1")
        nc.vector.memset(v1[:, :, :, 64:65], 1.0)
        qTs, kTs = [], []
        for u in range(NPAIR):
            qTs.append(qkpool.tile([128, S], BF16, tag=f"qT{u}", name=f"qT{u}"))
            kTs.append(qkpool.tile([128, S], BF16, tag=f"kT{u}", name=f"kT{u}"))
        oTs = [otpool.tile([128, S], BF16, tag=f"oT{u}", name=f"oT{u}")
               for u in range(NPAIR)]
        drpool = ep(tc.tile_pool(name="dr", bufs=4, space="DRAM"))
        dnrowpool = ep(tc.tile_pool(name="dnrow", bufs=2))
        dnBpool = ep(tc.tile_pool(name="dnB", bufs=2))
        onesc = consts.tile([64, 1024], F32, tag="onesc")
        nc.vector.memset(onesc, 1.0)

        # ---------------- emission helpers ----------------
        def vproj_chunk(sb, fine=False):
            """one sb block (128 seq positions) of the V projection"""
            aux = auxpool.tile([128, 512], F32, tag="aux")
            for ic in range(8):
                nc.tensor.matmul(aux, xts[ic][:, sb * 128:(sb + 1) * 128],
                                 wv[ic], start=(ic == 0), stop=(ic == 7),
                                 skip_group_check=True)
            nc.vector.tensor_copy(
                out=v1[:, sb, :, 0:64],
                in_=aux.rearrange("p (h d) -> p h d", d=64))
            if fine:
                yield

        def qkproj_unit(u, pj, sc):
            """one (proj, 512-col seq chunk): 8 matmuls + RoPE"""
            wlist, dst = ((wq, qTs[u]) if pj == 0 else (wk, kTs[u]))
            ssl = slice(sc * 512, (sc + 1) * 512)
            aux = auxpool.tile([128, 512], F32, tag="aux")
            for ic in range(8):
                nc.tensor.matmul(aux, wlist[ic][:, u * 128:(u + 1) * 128],
                                 xts[ic][:, ssl], start=(ic == 0),
                                 stop=(ic == 7), skip_group_check=True)
                if ic in (1, 3, 5):
                    yield
            sh = shpool.tile([128, 512], F32, tag="sh")
            nc.vector.stream_shuffle(out=sh, in_=aux, mask=swapmask)
            t1 = t1pool.tile([128, 512], BF16, tag="t1")
            nc.vector.tensor_mul(out=t1, in0=aux, in1=ctab[:, ssl])
            t2 = t2pool.tile([128, 512], BF16, tag="t2")
            nc.gpsimd.tensor_mul(out=t2, in0=sh, in1=stab[:, ssl])
            nc.gpsimd.tensor_add(out=dst[:, ssl], in0=t1, in1=t2)
            yield

        def qkproj_gen(u):
            for sc in range(4):
                for pj in range(2):
                    yield from qkproj_unit(u, pj, sc)

        def vproj_gen(sb0, sb1):
            for sb in range(sb0, sb1):
                yield from vproj_chunk(sb, fine=True)

        def oproj_gen(qc):
            """output projection for q chunk qc: 8 (ot,half) groups"""
            qsl = slice(qc * QC, (qc + 1) * QC)
            for oc in range(8):
                po = auxpool.tile([128, 512], F32, tag="aux")
                for u in range(NPAIR):
                    nc.tensor.matmul(
                        po, wo[u][:, oc * 128:(oc + 1) * 128],
                        oTs[u][:, qsl], start=(u == 0), stop=(u == NPAIR - 1),
                        skip_group_check=True)
                    if u == 1:
                        yield
                ob = obpool.tile([128, 512], F32, tag="ob")
                nc.vector.tensor_copy(out=ob, in_=po)
                nc.sync.dma_start(out=out_d[oc * 128:(oc + 1) * 128, qsl],
                                  in_=ob)
                yield

        def attn_chunk(u, qc, filler, npull=1):
            nact = 4 * qc + 4
            qbase = qc * QC
            pso = psopool.tile([128, 1024], F32, tag="pso")
            prev = None

            def emit_av(kc, pq):
                off = max(kc - 4 * qc, 0) * 128
                for h in range(2):
                    nc.tensor.matmul(
                        pso[0:65, h * 512 + off:(h + 1) * 512],
                        v1[:, kc, 2 * u + h, :],
                        pq[:, h * 512 + off:(h + 1) * 512],
                        start=(kc == 0), stop=(kc == nact - 1),
                        skip_group_check=True)

            for kc in range(nact):
                moff = kc - 4 * qc
                off = max(moff, 0) * 128
                psq = psqpool.tile([128, 1024], F32, tag="psq")
                for h in range(2):
                    nc.tensor.matmul(
                        psq[:, h * 512 + off:(h + 1) * 512],
                        kTs[u][h * 64:(h + 1) * 64, kc * 128:(kc + 1) * 128],
                        qTs[u][h * 64:(h + 1) * 64, qbase + off:qbase + 512],
                        start=True, stop=True, skip_group_check=True)
                psq3 = psq.rearrange("p (h q) -> p h q", q=512)
                if moff >= 0:
                    nc.vector.tensor_add(out=psq3[:, :, off:off + 128],
                                         in0=psq3[:, :, off:off + 128],
                                         in1=trineg3)
                pq = pqpool.tile([128, 1024], BF16, tag="pq")
                pq3 = pq.rearrange("p (h q) -> p h q", q=512)
                nc.scalar.activation(out=pq3[:, :, off:], in_=psq3[:, :, off:],
                                     func=EXP, scale=0.125)
                if prev is not None:
                    emit_av(*prev)
                for _ in range(npull):
                    next(filler, None)
                prev = (kc, pq)
            emit_av(*prev)
            # cross-partition moves must be TensorTensor ops (BIR verifier
            # rejects TensorCopy with mismatched start partitions); gpsimd
            # carries these so the vector engine keeps slack
            for h in range(2):
                nc.vector.tensor_mul(
                    out=oTs[u][h * 64:(h + 1) * 64, qbase:qbase + 512],
                    in0=pso[0:64, h * 512:(h + 1) * 512],
                    in1=onesc[:, 0:512])
            dnrow = dnrowpool.tile([1, 1024], F32, tag="dnrow")
            nc.vector.tensor_mul(out=dnrow, in0=pso[64:65, :],
                                 in1=onesc[0:1, :])
            # distributed softmax denominator: bounce the dn row through
            # DRAM to broadcast it across partitions, then reciprocal + two
            # in-place normalize multiplies.  No cross-u barrier.
            drt = drpool.tile([1, 1024], F32)
            nc.sync.dma_start(out=drt, in_=dnrow)
            dnB = dnBpool.tile([128, 1024], F32, tag="dnB")
            nc.sync.dma_start(out=dnB, in_=drt[0:1, :].to_broadcast((128, 1024)))
            recB = recBpool.tile([128, 1024], F32, tag="recB")
            nc.vector.reciprocal_approx_fast(out=recB, in_=dnB)
            for h in range(2):
                hs = slice(h * 64, (h + 1) * 64)
                nc.gpsimd.tensor_mul(
                    out=oTs[u][hs, qbase:qbase + 512],
                    in0=oTs[u][hs, qbase:qbase + 512],
                    in1=recB[hs, h * 512:(h + 1) * 512])

        def chain(*gens):
            for g in gens:
                yield from g

        def drain(g):
            for _ in g:
                pass

        # ---------------- schedule ----------------
        _sc = nc.named_scope("qkproj0"); _sc.__enter__()
        for sc in range(4):
            for pj in range(2):
                drain(qkproj_unit(0, pj, sc))
            if sc == 0:
                for sb in range(4):
                    for _ in vproj_chunk(sb):
                        pass
        _sc.__exit__(None, None, None)

        fillers = [chain(vproj_gen(4, NKC), qkproj_gen(1)),
                   qkproj_gen(2), qkproj_gen(3)]
        for u in range(NPAIR):
            _sc = nc.named_scope(f"stripe{u}"); _sc.__enter__()
            if u < 3:
                filler = fillers[u]
                for qc in range(NQC):
                    attn_chunk(u, qc, filler)
                drain(filler)
            else:
                filler = iter(())
                for qc in range(NQC):
                    attn_chunk(3, qc, filler)
                    if qc < NQC - 1:
                        filler = chain(filler, oproj_gen(qc))
                drain(filler)
            _sc.__exit__(None, None, None)
        _sc = nc.named_scope("oproj3"); _sc.__enter__()
        # final output projection: attention is done, so the psq pool (4
        # banks) and the scalar engine are free -- use wide po tiles and
        # ACT-engine copies so the groups pipeline without V-queue gating
        qsl = slice((NQC - 1) * QC, NQC * QC)
        for ot in range(4):
            po = psqpool.tile([128, 1024], F32, tag="psq")
            for half in range(2):
                oc = 2 * ot + half
                for u in range(NPAIR):
                    nc.tensor.matmul(
                        po[:, half * 512:(half + 1) * 512],
                        wo[u][:, oc * 128:(oc + 1) * 128],
                        oTs[u][:, qsl], start=(u == 0), stop=(u == NPAIR - 1),
                        skip_group_check=True)
            obw = obpool.tile([128, 1024], F32, tag="obw", name=f"obw{ot}")
            nc.scalar.copy(out=obw, in_=po)
            for half in range(2):
                oc = 2 * ot + half
                nc.sync.dma_start(
                    out=out_d[oc * 128:(oc + 1) * 128, qsl],
                    in_=obw[:, half * 512:(half + 1) * 512])
        _sc.__exit__(None, None, None)

    nc.compile()
    return nc


def get_nc():
    if "nc" not in _BUILT:
        _BUILT["nc"] = _build_nc()
    return _BUILT["nc"]


def _host_prep(x, Wq, Wk, Wv, Wo, token_positions):
    pos = np.asarray(token_positions).astype(np.float32)
    half = DK // 2
    inv_freq = 1.0 / (10000.0 ** (np.arange(half, dtype=np.float32) * 2.0 / DK))
    ang = pos[:, None] * inv_freq[None, :]          # [S, 32]
    cos = np.cos(ang).astype(np.float32)            # [S, 32]
    sin = np.sin(ang).astype(np.float32)
    p = np.arange(128)
    j = (p % 64) // 2
    sign = np.where(p % 2 == 0, -1.0, 1.0).astype(np.float32)
    bf = ml_dtypes.bfloat16
    ctab = np.ascontiguousarray(cos[:, j].T).astype(bf)           # [128, S]
    stab = np.ascontiguousarray(sin[:, j].T * sign[:, None]).astype(bf)

    kk = np.arange(128)[:, None]
    qq = np.arange(128)[None, :]
    tri = np.where(qq >= kk, 0.0, -1e9).astype(np.float32)        # [128,128]
    trineg = np.concatenate([tri, tri], axis=1)                   # [128,256]

    in_maps = []
    for c in range(NCORES):
        b, hf = divmod(c, 2)
        m = {}
        m["xT"] = np.ascontiguousarray(x[b].T).astype(bf)
        m["wqT"] = np.ascontiguousarray(Wq[hf * 512:(hf + 1) * 512, :].T).astype(bf)
        m["wkT"] = np.ascontiguousarray(Wk[hf * 512:(hf + 1) * 512, :].T).astype(bf)
        m["wvT"] = np.ascontiguousarray(Wv[hf * 512:(hf + 1) * 512, :].T).astype(bf)
        m["woT"] = np.ascontiguousarray(Wo[:, hf * 512:(hf + 1) * 512].T).astype(bf)
        m["ctab"] = ctab
        m["stab"] = stab
        m["trineg"] = trineg
        in_maps.append(m)
    return in_maps


def run(inputs, trace=False, **kw):
    in_maps = _host_prep(**{k: np.asarray(v) for k, v in inputs.items()})
    nc = get_nc()
    res = run_bass_kernel_spmd(nc, in_maps, list(range(NCORES)), trace=trace, **kw)
    outs = [res.results[c]["outP"] for c in range(NCORES)]
    out = np.stack([(outs[2 * b] + outs[2 * b + 1]).T for b in range(B)])
    return out.astype(np.float32), res


def kernel(**inputs):
    out, _ = run(inputs, trace=False)
    return out
